# revision 2
# baseline (speedup 1.0000x reference)
"""Submanifold sparse 3D conv (160^3 grid, 400k voxels, 32->64ch, 3x3x3) on 8 trn2 cores.

Strategy (per sharding hint): voxels sharded by z-slab (20 planes/core), weights
replicated. Host does the sharding prep: sorts voxels by (z,y,x), builds the
per-device dense index grid lookups as per-window int16 slot tables, and packs
per-core transposed feature slabs with 1-plane halos. The device kernel streams
5-plane feature windows through SBUF, gathers neighbor features on-chip
(GPSIMD ap_gather), and accumulates 27 kernel-offset GEMMs (grouped 4-wide into
K=128 float32r matmuls) into PSUM.
"""

import sys

for _p in ("/opt/trn_rl_repo",):
    if _p not in sys.path:
        sys.path.insert(0, _p)

import numpy as np

# ---- problem constants (hardcoded; kernel.py must be self-contained) ----
D = H = W = 160
N_VOX = 400_000
C_IN, C_OUT = 32, 64
CORES = 8
ZPC = D // CORES  # 20 z-planes per core

# ---- tiling constants ----
OPW = 3                    # output planes per window
NWIN = -(-ZPC // OPW)      # 7 windows per core
WINP = OPW + 2             # planes per window (with z halo)
P_PAD = 2816               # padded rows per plane
WIN_ELE = WINP * P_PAD     # 14080
ZSLOT = WIN_ELE            # index of the always-zero slot
WIN_FREE = WIN_ELE + 64    # 14144, zero-padded tail
TILE = 512                 # voxels per matmul tile
NG = 7                     # offset groups (27 offsets in groups of 4)

_OFFSETS = [(dz, dy, dx) for dz in (-1, 0, 1) for dy in (-1, 0, 1) for dx in (-1, 0, 1)]

_PROG_CACHE = {}
LAST_RESULTS = None
TRACE = False


def _build_program(tpw):
    import concourse.bacc as bacc
    import concourse.tile as tile
    import concourse.mybir as mybir
    from contextlib import ExitStack

    dt = mybir.dt
    nc = bacc.Bacc("TRN2", target_bir_lowering=False, debug=False, num_devices=CORES)

    featw = nc.dram_tensor("featw", [NWIN, C_IN, WIN_FREE], dt.float32, kind="ExternalInput").ap()
    idx = nc.dram_tensor("idx", [NWIN, 128, tpw * NG * 32], dt.int16, kind="ExternalInput").ap()
    wts = nc.dram_tensor("wts", [128, NG * 64], dt.bfloat16, kind="ExternalInput").ap()
    bias = nc.dram_tensor("bias", [C_OUT, 1], dt.float32, kind="ExternalInput").ap()
    out = nc.dram_tensor("out", [C_OUT, NWIN * tpw * TILE], dt.float32, kind="ExternalOutput").ap()

    with tile.TileContext(nc) as tc, ExitStack() as ctx:
        consts = ctx.enter_context(tc.tile_pool(name="consts", bufs=1))
        winp = ctx.enter_context(tc.tile_pool(name="win", bufs=2))
        idxp = ctx.enter_context(tc.tile_pool(name="idxp", bufs=2))
        xp = ctx.enter_context(tc.tile_pool(name="x", bufs=4))
        xbp = ctx.enter_context(tc.tile_pool(name="xb", bufs=4))
        pp = ctx.enter_context(tc.tile_pool(name="psum", bufs=4, space="PSUM"))
        op = ctx.enter_context(tc.tile_pool(name="outp", bufs=4))

        wsb = consts.tile([128, NG * 64], dt.bfloat16)
        nc.sync.dma_start(wsb[:], wts[:])
        bsb = consts.tile([C_OUT, 1], dt.float32)
        nc.sync.dma_start(bsb[:], bias[:])

        for w in range(NWIN):
            win = winp.tile([128, WIN_FREE], dt.float32)
            for q in range(4):
                nc.sync.dma_start(win[32 * q:32 * (q + 1), :], featw[w])
            ix = idxp.tile([128, tpw * NG * 32], dt.int16)
            nc.sync.dma_start(ix[:], idx[w])
            for t in range(tpw):
                ps = pp.tile([C_OUT, TILE], dt.float32)
                for g in range(NG):
                    x = xp.tile([128, TILE], dt.float32)
                    col = (t * NG + g) * 32
                    nc.gpsimd.ap_gather(
                        x[:], win[:], ix[:, col:col + 32],
                        channels=128, num_elems=WIN_FREE, d=1, num_idxs=TILE,
                    )
                    xb = xbp.tile([128, TILE], dt.bfloat16)
                    nc.scalar.copy(xb[:], x[:])
                    nc.tensor.matmul(
                        ps[:], wsb[:, g * 64:(g + 1) * 64], xb[:],
                        start=(g == 0), stop=(g == NG - 1),
                    )
                ot = op.tile([C_OUT, TILE], dt.float32)
                nc.vector.tensor_scalar_add(ot[:], ps[:], bsb[:])
                c0 = (w * tpw + t) * TILE
                nc.sync.dma_start(out[:, c0:c0 + TILE], ot[:])

    nc.compile()
    return nc


def _prep(features, coors, weight, bias):
    feats = np.asarray(features, np.float32)
    co = np.asarray(coors, np.int32)
    wt = np.asarray(weight, np.float32)
    bi = np.asarray(bias, np.float32)
    n = feats.shape[0]
    assert n == N_VOX, n

    z = co[:, 1].astype(np.int64)
    y = co[:, 2].astype(np.int64)
    x = co[:, 3].astype(np.int64)
    p = (z * H + y) * W + x
    perm = np.argsort(p, kind="stable")
    ps_ = p[perm]
    zs = (ps_ // (H * W)).astype(np.int64)
    ys = (ps_ // W) % H
    xs = ps_ % W
    fsT = np.ascontiguousarray(feats[perm].T)  # [32, N]

    grid = np.full(D * H * W, -1, np.int32)
    grid[ps_] = np.arange(n, dtype=np.int32)

    pstart = np.searchsorted(zs, np.arange(D + 1)).astype(np.int64)  # [161]
    plane_rows = np.diff(pstart)
    assert plane_rows.max() <= P_PAD, plane_rows.max()
    rip = np.arange(n, dtype=np.int64) - pstart[zs]  # row index within plane

    # neighbor global (sorted) row per offset, -1 if absent
    nbr = np.empty((27, n), np.int32)
    for k, (dz, dy, dx) in enumerate(_OFFSETS):
        nz, ny, nx = zs + dz, ys + dy, xs + dx
        inb = (nz >= 0) & (nz < D) & (ny >= 0) & (ny < H) & (nx >= 0) & (nx < W)
        q = np.clip((nz * H + ny) * W + nx, 0, D * H * W - 1)
        nbr[k] = np.where(inb, grid[q], -1)

    # window row ranges per (core, window)
    win_meta = np.zeros((CORES, NWIN, 2), np.int64)  # (row_lo, n_rows)
    tpw = 1
    for c in range(CORES):
        for w in range(NWIN):
            zlo = c * ZPC + w * OPW
            zhi = min(zlo + OPW, (c + 1) * ZPC)
            r0, r1 = pstart[zlo], pstart[zhi]
            win_meta[c, w] = (r0, r1 - r0)
            tpw = max(tpw, -(-(r1 - r0) // TILE))
    tpw = int(tpw)

    featw = np.zeros((CORES, NWIN, C_IN, WIN_FREE), np.float32)
    for c in range(CORES):
        for w in range(NWIN):
            base = c * ZPC + w * OPW - 1
            for s in range(WINP):
                zz = base + s
                if 0 <= zz < D:
                    nr = plane_rows[zz]
                    featw[c, w, :, s * P_PAD:s * P_PAD + nr] = fsT[:, pstart[zz]:pstart[zz + 1]]

    idxarr = np.full((CORES, NWIN, 128, tpw, NG, 32), ZSLOT, np.int16)
    zs32 = zs.astype(np.int64)
    for c in range(CORES):
        for w in range(NWIN):
            r0, nr = win_meta[c, w]
            base = c * ZPC + w * OPW - 1
            for g in range(NG):
                for q in range(4):
                    a = 4 * g + q
                    if a >= 27:
                        continue
                    gl = nbr[a, r0:r0 + nr].astype(np.int64)
                    gg = np.clip(gl, 0, n - 1)
                    slot = np.where(gl >= 0, (zs32[gg] - base) * P_PAD + rip[gg], ZSLOT)
                    assert slot.min() >= 0 and slot.max() <= ZSLOT
                    vals = np.full(tpw * TILE, ZSLOT, np.int64)
                    vals[:nr] = slot
                    blk = vals.reshape(tpw, 32, 16).transpose(2, 0, 1).astype(np.int16)
                    idxarr[c, w, 32 * q:32 * q + 16, :, g, :] = blk
                    idxarr[c, w, 32 * q + 16:32 * q + 32, :, g, :] = blk

    import ml_dtypes
    wts = np.zeros((128, NG * 64), np.float32)
    for g in range(NG):
        for q in range(4):
            a = 4 * g + q
            if a < 27:
                wts[32 * q:32 * q + 32, 64 * g:64 * g + 64] = wt[a]

    in_maps = [
        {
            "featw": featw[c],
            "idx": np.ascontiguousarray(idxarr[c].reshape(NWIN, 128, tpw * NG * 32)),
            "wts": wts.astype(ml_dtypes.bfloat16),
            "bias": bi.reshape(C_OUT, 1),
        }
        for c in range(CORES)
    ]
    return in_maps, tpw, win_meta, perm


def _assemble(results, tpw, win_meta, perm):
    out_sorted = np.empty((N_VOX, C_OUT), np.float32)
    for c in range(CORES):
        oc = results[c]["out"]  # [64, NWIN*tpw*TILE]
        for w in range(NWIN):
            r0, nr = win_meta[c, w]
            c0 = w * tpw * TILE
            out_sorted[r0:r0 + nr] = oc[:, c0:c0 + nr].T
    final = np.empty((N_VOX, C_OUT), np.float32)
    final[perm] = out_sorted
    return final


def kernel(features, coors, weight, bias, batch_size=1, **_kw):
    global LAST_RESULTS
    from concourse.bass_utils import run_bass_kernel_spmd

    in_maps, tpw, win_meta, perm = _prep(features, coors, weight, bias)
    if tpw not in _PROG_CACHE:
        _PROG_CACHE[tpw] = _build_program(tpw)
    nc = _PROG_CACHE[tpw]
    br = run_bass_kernel_spmd(nc, in_maps, list(range(CORES)), trace=TRACE)
    LAST_RESULTS = br
    return _assemble(br.results, tpw, win_meta, perm)


# revision 3
# speedup vs baseline: 1.7338x; 1.7338x over previous
"""Submanifold sparse 3D conv (160^3 grid, 400k voxels, 32->64ch, 3x3x3) on 8 trn2 cores.

Strategy (per sharding hint): voxels sharded by z-slab (20 planes/core), weights
replicated. Host does the sharding prep: sorts voxels by (z,y,x), builds the
per-device dense index grid lookups as per-window int16 slot tables, and packs
per-core transposed feature slabs with 1-plane halos. The device kernel streams
5-plane feature windows through SBUF (bf16 channel-pairs packed in uint32, one
replica per 16-partition GPSIMD band = 8 kernel offsets per gather), gathers
neighbor features on-chip (GPSIMD ap_gather), and accumulates the 27
kernel-offset GEMMs into PSUM via even/odd-channel bf16 matmuls.
"""

import sys

for _p in ("/opt/trn_rl_repo",):
    if _p not in sys.path:
        sys.path.insert(0, _p)

import numpy as np

# ---- problem constants (hardcoded; kernel.py must be self-contained) ----
D = H = W = 160
N_VOX = 400_000
C_IN, C_OUT = 32, 64
CORES = 8
ZPC = D // CORES  # 20 z-planes per core

# ---- tiling constants ----
OPW = 3                    # output planes per window
NWIN = -(-ZPC // OPW)      # 7 windows per core
WINP = OPW + 2             # planes per window (with z halo)
P_PAD = 2816               # padded rows per plane
WIN_ELE = WINP * P_PAD     # 14080
ZSLOT = WIN_ELE            # index of the always-zero slot
WIN_FREE = WIN_ELE + 64    # 14144, zero-padded tail
TILE = 512                 # voxels per matmul tile
NG = 4                     # gather groups (27 offsets in groups of 8)

_OFFSETS = [(dz, dy, dx) for dz in (-1, 0, 1) for dy in (-1, 0, 1) for dx in (-1, 0, 1)]

_PROG_CACHE = {}
LAST_RESULTS = None
TRACE = False


def _build_program(tpw):
    import concourse.bacc as bacc
    import concourse.tile as tile
    import concourse.mybir as mybir
    from contextlib import ExitStack

    dt = mybir.dt
    nc = bacc.Bacc("TRN2", target_bir_lowering=False, debug=False, num_devices=CORES)

    featw = nc.dram_tensor("featw", [NWIN, 16, WIN_FREE], dt.uint32, kind="ExternalInput").ap()
    idx = nc.dram_tensor("idx", [NWIN, 128, tpw * NG * 32], dt.int16, kind="ExternalInput").ap()
    wtse = nc.dram_tensor("wtse", [128, NG * 64], dt.bfloat16, kind="ExternalInput").ap()
    wtso = nc.dram_tensor("wtso", [128, NG * 64], dt.bfloat16, kind="ExternalInput").ap()
    bias = nc.dram_tensor("bias", [C_OUT, 1], dt.float32, kind="ExternalInput").ap()
    out = nc.dram_tensor("out", [C_OUT, NWIN * tpw * TILE], dt.float32, kind="ExternalOutput").ap()

    with tile.TileContext(nc) as tc, ExitStack() as ctx:
        consts = ctx.enter_context(tc.tile_pool(name="consts", bufs=1))
        winp = ctx.enter_context(tc.tile_pool(name="win", bufs=2))
        idxp = ctx.enter_context(tc.tile_pool(name="idxp", bufs=2))
        xp = ctx.enter_context(tc.tile_pool(name="x", bufs=4))
        pp = ctx.enter_context(tc.tile_pool(name="psum", bufs=4, space="PSUM"))
        op = ctx.enter_context(tc.tile_pool(name="outp", bufs=4))

        wse = consts.tile([128, NG * 64], dt.bfloat16)
        nc.sync.dma_start(wse[:], wtse[:])
        wso = consts.tile([128, NG * 64], dt.bfloat16)
        nc.sync.dma_start(wso[:], wtso[:])
        bsb = consts.tile([C_OUT, 1], dt.float32)
        nc.sync.dma_start(bsb[:], bias[:])

        for w in range(NWIN):
            win = winp.tile([128, WIN_FREE], dt.uint32)
            for r in range(8):
                nc.sync.dma_start(win[16 * r:16 * (r + 1), :], featw[w])
            ix = idxp.tile([128, tpw * NG * 32], dt.int16)
            nc.sync.dma_start(ix[:], idx[w])
            for t in range(tpw):
                ps = pp.tile([C_OUT, TILE], dt.float32)
                for g in range(NG):
                    x2 = xp.tile([128, TILE], dt.uint32)
                    col = (t * NG + g) * 32
                    nc.gpsimd.ap_gather(
                        x2[:], win[:], ix[:, col:col + 32],
                        channels=128, num_elems=WIN_FREE, d=1, num_idxs=TILE,
                    )
                    xb = x2[:].bitcast(dt.bfloat16).rearrange("p (n two) -> p two n", two=2)
                    nc.tensor.matmul(
                        ps[:], wse[:, g * 64:(g + 1) * 64], xb[:, 0, :],
                        start=(g == 0), stop=False,
                    )
                    nc.tensor.matmul(
                        ps[:], wso[:, g * 64:(g + 1) * 64], xb[:, 1, :],
                        start=False, stop=(g == NG - 1),
                    )
                ot = op.tile([C_OUT, TILE], dt.float32)
                nc.vector.tensor_scalar_add(ot[:], ps[:], bsb[:])
                c0 = (w * tpw + t) * TILE
                nc.sync.dma_start(out[:, c0:c0 + TILE], ot[:])

    nc.compile()
    return nc


def _prep(features, coors, weight, bias):
    import ml_dtypes

    feats = np.asarray(features, np.float32)
    co = np.asarray(coors, np.int32)
    wt = np.asarray(weight, np.float32)
    bi = np.asarray(bias, np.float32)
    n = feats.shape[0]
    assert n == N_VOX, n

    z = co[:, 1].astype(np.int64)
    y = co[:, 2].astype(np.int64)
    x = co[:, 3].astype(np.int64)
    p = (z * H + y) * W + x
    perm = np.argsort(p, kind="stable")
    ps_ = p[perm]
    zs = (ps_ // (H * W)).astype(np.int64)
    ys = (ps_ // W) % H
    xs = ps_ % W

    # bf16 channel pairs packed into uint32: u32[v, cp] = bf16(ch 2cp) | bf16(ch 2cp+1)<<16
    fb = feats[perm].astype(ml_dtypes.bfloat16).view(np.uint16)  # [N, 32] u16
    fu32 = fb[:, 0::2].astype(np.uint32) | (fb[:, 1::2].astype(np.uint32) << 16)  # [N, 16]
    fu32T = np.ascontiguousarray(fu32.T)  # [16, N]

    grid = np.full(D * H * W, -1, np.int32)
    grid[ps_] = np.arange(n, dtype=np.int32)

    pstart = np.searchsorted(zs, np.arange(D + 1)).astype(np.int64)  # [161]
    plane_rows = np.diff(pstart)
    assert plane_rows.max() <= P_PAD, plane_rows.max()
    rip = np.arange(n, dtype=np.int64) - pstart[zs]  # row index within plane

    nbr = np.empty((27, n), np.int32)
    for k, (dz, dy, dx) in enumerate(_OFFSETS):
        nz, ny, nx = zs + dz, ys + dy, xs + dx
        inb = (nz >= 0) & (nz < D) & (ny >= 0) & (ny < H) & (nx >= 0) & (nx < W)
        q = np.clip((nz * H + ny) * W + nx, 0, D * H * W - 1)
        nbr[k] = np.where(inb, grid[q], -1)

    win_meta = np.zeros((CORES, NWIN, 2), np.int64)  # (row_lo, n_rows)
    tpw = 1
    for c in range(CORES):
        for w in range(NWIN):
            zlo = c * ZPC + w * OPW
            zhi = min(zlo + OPW, (c + 1) * ZPC)
            r0, r1 = pstart[zlo], pstart[zhi]
            win_meta[c, w] = (r0, r1 - r0)
            tpw = max(tpw, -(-(r1 - r0) // TILE))
    tpw = int(tpw)

    featw = np.zeros((CORES, NWIN, 16, WIN_FREE), np.uint32)
    for c in range(CORES):
        for w in range(NWIN):
            base = c * ZPC + w * OPW - 1
            for s in range(WINP):
                zz = base + s
                if 0 <= zz < D:
                    nr = plane_rows[zz]
                    featw[c, w, :, s * P_PAD:s * P_PAD + nr] = fu32T[:, pstart[zz]:pstart[zz + 1]]

    # idx: band a' (partitions 16a'..16a'+15) of group g = offset 8g+a'
    idxarr = np.full((CORES, NWIN, 128, tpw, NG, 32), ZSLOT, np.int16)
    for c in range(CORES):
        for w in range(NWIN):
            r0, nr = win_meta[c, w]
            base = c * ZPC + w * OPW - 1
            for g in range(NG):
                for a in range(8):
                    k = 8 * g + a
                    if k >= 27:
                        continue
                    gl = nbr[k, r0:r0 + nr].astype(np.int64)
                    gg = np.clip(gl, 0, n - 1)
                    slot = np.where(gl >= 0, (zs[gg] - base) * P_PAD + rip[gg], ZSLOT)
                    assert slot.min() >= 0 and slot.max() <= ZSLOT
                    vals = np.full(tpw * TILE, ZSLOT, np.int64)
                    vals[:nr] = slot
                    blk = vals.reshape(tpw, 32, 16).transpose(2, 0, 1).astype(np.int16)
                    idxarr[c, w, 16 * a:16 * a + 16, :, g, :] = blk

    # weights: wse[16a+cp, 64g+m] = w[8g+a, 2cp, m]; wso -> 2cp+1
    wts_e = np.zeros((128, NG * 64), np.float32)
    wts_o = np.zeros((128, NG * 64), np.float32)
    for g in range(NG):
        for a in range(8):
            k = 8 * g + a
            if k >= 27:
                continue
            wts_e[16 * a:16 * a + 16, 64 * g:64 * g + 64] = wt[k, 0::2, :]
            wts_o[16 * a:16 * a + 16, 64 * g:64 * g + 64] = wt[k, 1::2, :]

    in_maps = [
        {
            "featw": featw[c],
            "idx": np.ascontiguousarray(idxarr[c].reshape(NWIN, 128, tpw * NG * 32)),
            "wtse": wts_e.astype(ml_dtypes.bfloat16),
            "wtso": wts_o.astype(ml_dtypes.bfloat16),
            "bias": bi.reshape(C_OUT, 1),
        }
        for c in range(CORES)
    ]
    return in_maps, tpw, win_meta, perm


def _assemble(results, tpw, win_meta, perm):
    out_sorted = np.empty((N_VOX, C_OUT), np.float32)
    for c in range(CORES):
        oc = results[c]["out"]  # [64, NWIN*tpw*TILE]
        for w in range(NWIN):
            r0, nr = win_meta[c, w]
            c0 = w * tpw * TILE
            out_sorted[r0:r0 + nr] = oc[:, c0:c0 + nr].T
    final = np.empty((N_VOX, C_OUT), np.float32)
    final[perm] = out_sorted
    return final


def kernel(features, coors, weight, bias, batch_size=1, **_kw):
    global LAST_RESULTS
    from concourse.bass_utils import run_bass_kernel_spmd

    in_maps, tpw, win_meta, perm = _prep(features, coors, weight, bias)
    if tpw not in _PROG_CACHE:
        _PROG_CACHE[tpw] = _build_program(tpw)
    nc = _PROG_CACHE[tpw]
    br = run_bass_kernel_spmd(nc, in_maps, list(range(CORES)), trace=TRACE)
    LAST_RESULTS = br
    return _assemble(br.results, tpw, win_meta, perm)


# revision 4
# speedup vs baseline: 1.9181x; 1.1063x over previous
"""Submanifold sparse 3D conv (160^3 grid, 400k voxels, 32->64ch, 3x3x3) on 8 trn2 cores.

Strategy (per sharding hint): voxels sharded by z-slab (20 planes/core), weights
replicated. Host does the sharding prep: sorts voxels by (z,y,x), builds the
per-device dense index grid lookups as per-window int16 slot tables, and packs
per-core transposed feature slabs with 1-plane halos. The device kernel streams
5-plane feature windows through SBUF (bf16 channel-pairs packed in uint32, one
replica per 16-partition GPSIMD band = 8 kernel offsets per gather), gathers
neighbor features on-chip (GPSIMD ap_gather), and accumulates the 27
kernel-offset GEMMs into PSUM via even/odd-channel bf16 matmuls.
"""

import sys

for _p in ("/opt/trn_rl_repo",):
    if _p not in sys.path:
        sys.path.insert(0, _p)

import numpy as np

# ---- problem constants (hardcoded; kernel.py must be self-contained) ----
D = H = W = 160
N_VOX = 400_000
C_IN, C_OUT = 32, 64
CORES = 8
ZPC = D // CORES  # 20 z-planes per core

# ---- tiling constants ----
OPW = 5                    # output planes per window
NWIN = -(-ZPC // OPW)      # 7 windows per core
WINP = OPW + 2             # planes per window (with z halo)
P_PAD = 2816               # padded rows per plane
WIN_ELE = WINP * P_PAD     # 14080
ZSLOT = WIN_ELE            # index of the always-zero slot
WIN_FREE = WIN_ELE + 64    # 14144, zero-padded tail
TILE = 512                 # voxels per matmul tile
NG = 4                     # gather groups (27 offsets in groups of 8)

_OFFSETS = [(dz, dy, dx) for dz in (-1, 0, 1) for dy in (-1, 0, 1) for dx in (-1, 0, 1)]

_PROG_CACHE = {}
LAST_RESULTS = None
TRACE = False


def _build_program(tpw):
    import concourse.bacc as bacc
    import concourse.tile as tile
    import concourse.mybir as mybir
    from contextlib import ExitStack

    dt = mybir.dt
    nc = bacc.Bacc("TRN2", target_bir_lowering=False, debug=False, num_devices=CORES)

    featw = nc.dram_tensor("featw", [NWIN, 16, WIN_FREE], dt.uint32, kind="ExternalInput").ap()
    idx = nc.dram_tensor("idx", [NWIN, 128, tpw * NG * 32], dt.int16, kind="ExternalInput").ap()
    wtse = nc.dram_tensor("wtse", [128, NG * 64], dt.bfloat16, kind="ExternalInput").ap()
    wtso = nc.dram_tensor("wtso", [128, NG * 64], dt.bfloat16, kind="ExternalInput").ap()
    bias = nc.dram_tensor("bias", [C_OUT, 1], dt.float32, kind="ExternalInput").ap()
    out = nc.dram_tensor("out", [C_OUT, NWIN * tpw * TILE], dt.float32, kind="ExternalOutput").ap()

    with tile.TileContext(nc) as tc, ExitStack() as ctx:
        consts = ctx.enter_context(tc.tile_pool(name="consts", bufs=1))
        winp = ctx.enter_context(tc.tile_pool(name="win", bufs=2))
        idxp = ctx.enter_context(tc.tile_pool(name="idxp", bufs=2))
        xp = ctx.enter_context(tc.tile_pool(name="x", bufs=4))
        pp = ctx.enter_context(tc.tile_pool(name="psum", bufs=4, space="PSUM"))
        op = ctx.enter_context(tc.tile_pool(name="outp", bufs=4))

        wse = consts.tile([128, NG * 64], dt.bfloat16)
        nc.sync.dma_start(wse[:], wtse[:])
        wso = consts.tile([128, NG * 64], dt.bfloat16)
        nc.sync.dma_start(wso[:], wtso[:])
        bsb = consts.tile([C_OUT, 1], dt.float32)
        nc.sync.dma_start(bsb[:], bias[:])

        for w in range(NWIN):
            win = winp.tile([128, WIN_FREE], dt.uint32)
            for r in range(8):
                nc.sync.dma_start(win[16 * r:16 * (r + 1), :], featw[w])
            ix = idxp.tile([128, tpw * NG * 32], dt.int16)
            nc.sync.dma_start(ix[:], idx[w])
            for t in range(tpw):
                ps = pp.tile([C_OUT, TILE], dt.float32)
                for g in range(NG):
                    x2 = xp.tile([128, TILE], dt.uint32)
                    col = (t * NG + g) * 32
                    nc.gpsimd.ap_gather(
                        x2[:], win[:], ix[:, col:col + 32],
                        channels=128, num_elems=WIN_FREE, d=1, num_idxs=TILE,
                    )
                    xb = x2[:].bitcast(dt.bfloat16).rearrange("p (n two) -> p two n", two=2)
                    nc.tensor.matmul(
                        ps[:], wse[:, g * 64:(g + 1) * 64], xb[:, 0, :],
                        start=(g == 0), stop=False,
                    )
                    nc.tensor.matmul(
                        ps[:], wso[:, g * 64:(g + 1) * 64], xb[:, 1, :],
                        start=False, stop=(g == NG - 1),
                    )
                ot = op.tile([C_OUT, TILE], dt.float32)
                nc.vector.tensor_scalar_add(ot[:], ps[:], bsb[:])
                c0 = (w * tpw + t) * TILE
                nc.sync.dma_start(out[:, c0:c0 + TILE], ot[:])

    nc.compile()
    return nc


def _prep(features, coors, weight, bias):
    import ml_dtypes

    feats = np.asarray(features, np.float32)
    co = np.asarray(coors, np.int32)
    wt = np.asarray(weight, np.float32)
    bi = np.asarray(bias, np.float32)
    n = feats.shape[0]
    assert n == N_VOX, n

    z = co[:, 1].astype(np.int64)
    y = co[:, 2].astype(np.int64)
    x = co[:, 3].astype(np.int64)
    p = (z * H + y) * W + x
    perm = np.argsort(p, kind="stable")
    ps_ = p[perm]
    zs = (ps_ // (H * W)).astype(np.int64)
    ys = (ps_ // W) % H
    xs = ps_ % W

    # bf16 channel pairs packed into uint32: u32[v, cp] = bf16(ch 2cp) | bf16(ch 2cp+1)<<16
    fb = feats[perm].astype(ml_dtypes.bfloat16).view(np.uint16)  # [N, 32] u16
    fu32 = fb[:, 0::2].astype(np.uint32) | (fb[:, 1::2].astype(np.uint32) << 16)  # [N, 16]
    fu32T = np.ascontiguousarray(fu32.T)  # [16, N]

    grid = np.full(D * H * W, -1, np.int32)
    grid[ps_] = np.arange(n, dtype=np.int32)

    pstart = np.searchsorted(zs, np.arange(D + 1)).astype(np.int64)  # [161]
    plane_rows = np.diff(pstart)
    assert plane_rows.max() <= P_PAD, plane_rows.max()
    rip = np.arange(n, dtype=np.int64) - pstart[zs]  # row index within plane

    nbr = np.empty((27, n), np.int32)
    for k, (dz, dy, dx) in enumerate(_OFFSETS):
        nz, ny, nx = zs + dz, ys + dy, xs + dx
        inb = (nz >= 0) & (nz < D) & (ny >= 0) & (ny < H) & (nx >= 0) & (nx < W)
        q = np.clip((nz * H + ny) * W + nx, 0, D * H * W - 1)
        nbr[k] = np.where(inb, grid[q], -1)

    win_meta = np.zeros((CORES, NWIN, 2), np.int64)  # (row_lo, n_rows)
    tpw = 1
    for c in range(CORES):
        for w in range(NWIN):
            zlo = c * ZPC + w * OPW
            zhi = min(zlo + OPW, (c + 1) * ZPC)
            r0, r1 = pstart[zlo], pstart[zhi]
            win_meta[c, w] = (r0, r1 - r0)
            tpw = max(tpw, -(-(r1 - r0) // TILE))
    tpw = int(tpw)

    featw = np.zeros((CORES, NWIN, 16, WIN_FREE), np.uint32)
    for c in range(CORES):
        for w in range(NWIN):
            base = c * ZPC + w * OPW - 1
            for s in range(WINP):
                zz = base + s
                if 0 <= zz < D:
                    nr = plane_rows[zz]
                    featw[c, w, :, s * P_PAD:s * P_PAD + nr] = fu32T[:, pstart[zz]:pstart[zz + 1]]

    # idx: band a' (partitions 16a'..16a'+15) of group g = offset 8g+a'
    idxarr = np.full((CORES, NWIN, 128, tpw, NG, 32), ZSLOT, np.int16)
    for c in range(CORES):
        for w in range(NWIN):
            r0, nr = win_meta[c, w]
            base = c * ZPC + w * OPW - 1
            for g in range(NG):
                for a in range(8):
                    k = 8 * g + a
                    if k >= 27:
                        continue
                    gl = nbr[k, r0:r0 + nr].astype(np.int64)
                    gg = np.clip(gl, 0, n - 1)
                    slot = np.where(gl >= 0, (zs[gg] - base) * P_PAD + rip[gg], ZSLOT)
                    assert slot.min() >= 0 and slot.max() <= ZSLOT
                    vals = np.full(tpw * TILE, ZSLOT, np.int64)
                    vals[:nr] = slot
                    blk = vals.reshape(tpw, 32, 16).transpose(2, 0, 1).astype(np.int16)
                    idxarr[c, w, 16 * a:16 * a + 16, :, g, :] = blk

    # weights: wse[16a+cp, 64g+m] = w[8g+a, 2cp, m]; wso -> 2cp+1
    wts_e = np.zeros((128, NG * 64), np.float32)
    wts_o = np.zeros((128, NG * 64), np.float32)
    for g in range(NG):
        for a in range(8):
            k = 8 * g + a
            if k >= 27:
                continue
            wts_e[16 * a:16 * a + 16, 64 * g:64 * g + 64] = wt[k, 0::2, :]
            wts_o[16 * a:16 * a + 16, 64 * g:64 * g + 64] = wt[k, 1::2, :]

    in_maps = [
        {
            "featw": featw[c],
            "idx": np.ascontiguousarray(idxarr[c].reshape(NWIN, 128, tpw * NG * 32)),
            "wtse": wts_e.astype(ml_dtypes.bfloat16),
            "wtso": wts_o.astype(ml_dtypes.bfloat16),
            "bias": bi.reshape(C_OUT, 1),
        }
        for c in range(CORES)
    ]
    return in_maps, tpw, win_meta, perm


def _assemble(results, tpw, win_meta, perm):
    out_sorted = np.empty((N_VOX, C_OUT), np.float32)
    for c in range(CORES):
        oc = results[c]["out"]  # [64, NWIN*tpw*TILE]
        for w in range(NWIN):
            r0, nr = win_meta[c, w]
            c0 = w * tpw * TILE
            out_sorted[r0:r0 + nr] = oc[:, c0:c0 + nr].T
    final = np.empty((N_VOX, C_OUT), np.float32)
    final[perm] = out_sorted
    return final


def kernel(features, coors, weight, bias, batch_size=1, **_kw):
    global LAST_RESULTS
    from concourse.bass_utils import run_bass_kernel_spmd

    in_maps, tpw, win_meta, perm = _prep(features, coors, weight, bias)
    if tpw not in _PROG_CACHE:
        _PROG_CACHE[tpw] = _build_program(tpw)
    nc = _PROG_CACHE[tpw]
    br = run_bass_kernel_spmd(nc, in_maps, list(range(CORES)), trace=TRACE)
    LAST_RESULTS = br
    return _assemble(br.results, tpw, win_meta, perm)


# revision 6
# speedup vs baseline: 2.5049x; 1.3059x over previous
"""Submanifold sparse 3D conv (160^3 grid, 400k voxels, 32->64ch, 3x3x3) on 8 trn2 cores.

Strategy (per sharding hint): voxels sharded by z-slab (20 planes/core), weights
replicated. Host does the sharding prep: sorts voxels by (z,y,x), builds the
per-device dense index grid lookups as per-window int16 slot tables, and packs
per-core feature windows (bf16 channel-pairs in uint32, one replica per
16-partition GPSIMD band). On device, 24 of the 27 kernel offsets are gathered
on-chip with ap_gather (8 offsets per call); the x-1/identity/x+1 offsets are
built on the Vector/Scalar engines as shifted window slices (sorted order makes
x-neighbors row-adjacent) with host-computed masks. All 27 offset GEMMs
accumulate in PSUM via even/odd-channel bf16 matmuls.
"""

import sys

for _p in ("/opt/trn_rl_repo",):
    if _p not in sys.path:
        sys.path.insert(0, _p)

import numpy as np

# ---- problem constants (hardcoded; kernel.py must be self-contained) ----
D = H = W = 160
N_VOX = 400_000
C_IN, C_OUT = 32, 64
CORES = 8
ZPC = D // CORES  # 20 z-planes per core

# ---- tiling constants ----
OPW = 5                    # output planes per window
NWIN = -(-ZPC // OPW)      # 4 windows per core
TILE = 512                 # voxels per matmul tile
NGG = 3                    # gather groups (24 gathered offsets, 8 per call)

_OFFSETS = [(dz, dy, dx) for dz in (-1, 0, 1) for dy in (-1, 0, 1) for dx in (-1, 0, 1)]
_GATHER_KS = [k for k in range(27) if k not in (12, 13, 14)]  # 24 offsets

_PROG_CACHE = {}
LAST_RESULTS = None
TRACE = False


def _build_program(tpw, win_free):
    import concourse.bacc as bacc
    import concourse.tile as tile
    import concourse.mybir as mybir
    from contextlib import ExitStack

    dt = mybir.dt
    nc = bacc.Bacc("TRN2", target_bir_lowering=False, debug=False, num_devices=CORES)

    featw = nc.dram_tensor("featw", [NWIN, 16, win_free], dt.uint32, kind="ExternalInput").ap()
    idx = nc.dram_tensor("idx", [NWIN, 128, tpw * NGG * 32], dt.int16, kind="ExternalInput").ap()
    msk = nc.dram_tensor("msk", [NWIN, tpw, 96, 2 * TILE], dt.bfloat16, kind="ExternalInput").ap()
    wtse = nc.dram_tensor("wtse", [128, NGG * 64], dt.bfloat16, kind="ExternalInput").ap()
    wtso = nc.dram_tensor("wtso", [128, NGG * 64], dt.bfloat16, kind="ExternalInput").ap()
    wce = nc.dram_tensor("wce", [96, 64], dt.bfloat16, kind="ExternalInput").ap()
    wco = nc.dram_tensor("wco", [96, 64], dt.bfloat16, kind="ExternalInput").ap()
    bias = nc.dram_tensor("bias", [C_OUT, 1], dt.float32, kind="ExternalInput").ap()
    out = nc.dram_tensor("out", [C_OUT, NWIN * tpw * TILE], dt.float32, kind="ExternalOutput").ap()

    with tile.TileContext(nc) as tc, ExitStack() as ctx:
        consts = ctx.enter_context(tc.tile_pool(name="consts", bufs=1))
        winp = ctx.enter_context(tc.tile_pool(name="win", bufs=2))
        idxp = ctx.enter_context(tc.tile_pool(name="idxp", bufs=2))
        mkp = ctx.enter_context(tc.tile_pool(name="mkp", bufs=3))
        xp = ctx.enter_context(tc.tile_pool(name="x", bufs=4))
        xcp = ctx.enter_context(tc.tile_pool(name="xc", bufs=3))
        pp = ctx.enter_context(tc.tile_pool(name="psum", bufs=4, space="PSUM"))
        op = ctx.enter_context(tc.tile_pool(name="outp", bufs=4))

        wse = consts.tile([128, NGG * 64], dt.bfloat16)
        nc.sync.dma_start(wse[:], wtse[:])
        wso = consts.tile([128, NGG * 64], dt.bfloat16)
        nc.sync.dma_start(wso[:], wtso[:])
        wcet = consts.tile([96, 64], dt.bfloat16)
        nc.sync.dma_start(wcet[:], wce[:])
        wcot = consts.tile([96, 64], dt.bfloat16)
        nc.sync.dma_start(wcot[:], wco[:])
        bsb = consts.tile([C_OUT, 1], dt.float32)
        nc.sync.dma_start(bsb[:], bias[:])

        for w in range(NWIN):
            win = winp.tile([128, win_free], dt.uint32)
            for r in range(8):
                nc.sync.dma_start(win[16 * r:16 * (r + 1), :], featw[w])
            winb = win[:].bitcast(dt.bfloat16)  # [128, 2*win_free]
            ix = idxp.tile([128, tpw * NGG * 32], dt.int16)
            nc.sync.dma_start(ix[:], idx[w])
            for t in range(tpw):
                ps = pp.tile([C_OUT, TILE], dt.float32)
                for g in range(NGG):
                    x2 = xp.tile([128, TILE], dt.uint32)
                    col = (t * NGG + g) * 32
                    nc.gpsimd.ap_gather(
                        x2[:], win[:], ix[:, col:col + 32],
                        channels=128, num_elems=win_free, d=1, num_idxs=TILE,
                    )
                    xb = x2[:].bitcast(dt.bfloat16).rearrange("p (n two) -> p two n", two=2)
                    nc.tensor.matmul(
                        ps[:], wse[:, g * 64:(g + 1) * 64], xb[:, 0, :],
                        start=(g == 0), stop=False,
                    )
                    nc.tensor.matmul(
                        ps[:], wso[:, g * 64:(g + 1) * 64], xb[:, 1, :],
                        start=False, stop=False,
                    )
                # cheap group: x-1 / identity / x+1 built from shifted slices
                mk = mkp.tile([96, 2 * TILE], dt.bfloat16)
                nc.sync.dma_start(mk[:], msk[w, t])
                xc = xcp.tile([96, 2 * TILE], dt.bfloat16)
                s0 = 1 + t * TILE  # slot of this tile's first voxel
                b0 = 2 * (s0 - 1)
                nc.vector.tensor_mul(xc[0:32, :], winb[0:32, b0:b0 + 2 * TILE], mk[0:32, :])
                nc.scalar.copy(xc[32:64, :], winb[32:64, b0 + 2:b0 + 2 + 2 * TILE])
                nc.vector.tensor_mul(xc[64:96, :], winb[64:96, b0 + 4:b0 + 4 + 2 * TILE], mk[64:96, :])
                xcb = xc[:].rearrange("p (n two) -> p two n", two=2)
                nc.tensor.matmul(ps[:], wcet[:], xcb[:, 0, :], start=False, stop=False)
                nc.tensor.matmul(ps[:], wcot[:], xcb[:, 1, :], start=False, stop=True)

                ot = op.tile([C_OUT, TILE], dt.float32)
                nc.vector.tensor_scalar_add(ot[:], ps[:], bsb[:])
                c0 = (w * tpw + t) * TILE
                nc.sync.dma_start(out[:, c0:c0 + TILE], ot[:])

    nc.compile()
    return nc


def _prep(features, coors, weight, bias):
    import ml_dtypes

    feats = np.asarray(features, np.float32)
    co = np.asarray(coors, np.int32)
    wt = np.asarray(weight, np.float32)
    bi = np.asarray(bias, np.float32)
    n = feats.shape[0]
    assert n == N_VOX, n

    z = co[:, 1].astype(np.int64)
    y = co[:, 2].astype(np.int64)
    x = co[:, 3].astype(np.int64)
    p = (z * H + y) * W + x
    perm = np.argsort(p, kind="stable")
    ps_ = p[perm]
    zs = (ps_ // (H * W)).astype(np.int64)
    ys = (ps_ // W) % H
    xs = ps_ % W

    # bf16 channel pairs packed into uint32
    fb = feats[perm].astype(ml_dtypes.bfloat16).view(np.uint16)  # [N, 32] u16
    fu32 = fb[:, 0::2].astype(np.uint32) | (fb[:, 1::2].astype(np.uint32) << 16)  # [N, 16]
    fu32T = np.ascontiguousarray(fu32.T)  # [16, N]

    grid = np.full(D * H * W, -1, np.int32)
    grid[ps_] = np.arange(n, dtype=np.int32)

    pstart = np.searchsorted(zs, np.arange(D + 1)).astype(np.int64)  # [161]

    nbr = np.empty((27, n), np.int32)
    for k, (dz, dy, dx) in enumerate(_OFFSETS):
        nz, ny, nx = zs + dz, ys + dy, xs + dx
        inb = (nz >= 0) & (nz < D) & (ny >= 0) & (ny < H) & (nx >= 0) & (nx < W)
        q = np.clip((nz * H + ny) * W + nx, 0, D * H * W - 1)
        nbr[k] = np.where(inb, grid[q], -1)

    # window geometry: out planes [zlo, zhi); window rows = planes [zlo, zhi+1)
    # packed first (slot 1+g-r0), then halo plane zlo-1 at the tail; slot 0 = zeros
    win_meta = np.zeros((CORES, NWIN, 5), np.int64)  # r0, n_out, r1x, h0, h1
    tpw, max_rows = 1, 0
    for c in range(CORES):
        for w in range(NWIN):
            zlo = c * ZPC + w * OPW
            zhi = min(zlo + OPW, (c + 1) * ZPC)
            r0, r1 = pstart[zlo], pstart[zhi]
            r1x = pstart[min(zhi + 1, D)]
            h0, h1 = (pstart[zlo - 1], r0) if zlo > 0 else (0, 0)
            win_meta[c, w] = (r0, r1 - r0, r1x, h0, h1)
            tpw = max(tpw, -(-(r1 - r0) // TILE))
            max_rows = max(max_rows, (r1x - r0) + (h1 - h0))
    tpw = int(tpw)
    win_free = int(-(-(2 + max(max_rows, 2 + tpw * TILE)) // 64) * 64)
    zslot = 0

    def slot_of(g, r0, r1x, h0):
        # g: global sorted row within the window's planes
        return np.where(g >= r0, 1 + g - r0, 1 + (r1x - r0) + (g - h0))

    featw = np.zeros((CORES, NWIN, 16, win_free), np.uint32)
    for c in range(CORES):
        for w in range(NWIN):
            r0, n_out, r1x, h0, h1 = win_meta[c, w]
            featw[c, w, :, 1:1 + (r1x - r0)] = fu32T[:, r0:r1x]
            if h1 > h0:
                featw[c, w, :, 1 + (r1x - r0):1 + (r1x - r0) + (h1 - h0)] = fu32T[:, h0:h1]

    idxarr = np.full((CORES, NWIN, 128, tpw, NGG, 32), zslot, np.int16)
    masks = np.zeros((CORES, NWIN, tpw, 96, 2 * TILE), ml_dtypes.bfloat16)
    for c in range(CORES):
        for w in range(NWIN):
            r0, n_out, r1x, h0, h1 = win_meta[c, w]
            for g in range(NGG):
                for a in range(8):
                    k = _GATHER_KS[8 * g + a]
                    gl = nbr[k, r0:r0 + n_out].astype(np.int64)
                    gg = np.clip(gl, 0, n - 1)
                    slot = np.where(gl >= 0, slot_of(gg, r0, r1x, h0), zslot)
                    assert slot.min() >= 0 and slot.max() < win_free
                    vals = np.full(tpw * TILE, zslot, np.int64)
                    vals[:n_out] = slot
                    blk = vals.reshape(tpw, 32, 16).transpose(2, 0, 1).astype(np.int16)
                    idxarr[c, w, 16 * a:16 * a + 16, :, g, :] = blk
            rows = np.arange(r0, r0 + n_out)
            m1 = np.zeros(tpw * TILE, np.float32)
            p1 = np.zeros(tpw * TILE, np.float32)
            m1[:n_out] = (nbr[12, r0:r0 + n_out] == rows - 1).astype(np.float32)
            p1[:n_out] = (nbr[14, r0:r0 + n_out] == rows + 1).astype(np.float32)
            m1d = np.repeat(m1.reshape(tpw, TILE), 2, axis=1).astype(ml_dtypes.bfloat16)
            p1d = np.repeat(p1.reshape(tpw, TILE), 2, axis=1).astype(ml_dtypes.bfloat16)
            masks[c, w, :, 0:16, :] = m1d[:, None, :]
            masks[c, w, :, 64:80, :] = p1d[:, None, :]

    wts_e = np.zeros((128, NGG * 64), np.float32)
    wts_o = np.zeros((128, NGG * 64), np.float32)
    for g in range(NGG):
        for a in range(8):
            k = _GATHER_KS[8 * g + a]
            wts_e[16 * a:16 * a + 16, 64 * g:64 * g + 64] = wt[k, 0::2, :]
            wts_o[16 * a:16 * a + 16, 64 * g:64 * g + 64] = wt[k, 1::2, :]
    wc_e = np.zeros((96, 64), np.float32)
    wc_o = np.zeros((96, 64), np.float32)
    for a, k in enumerate((12, 13, 14)):
        wc_e[32 * a:32 * a + 16] = wt[k, 0::2, :]
        wc_o[32 * a:32 * a + 16] = wt[k, 1::2, :]

    in_maps = [
        {
            "featw": featw[c],
            "idx": np.ascontiguousarray(idxarr[c].reshape(NWIN, 128, tpw * NGG * 32)),
            "msk": np.ascontiguousarray(masks[c]),
            "wtse": wts_e.astype(ml_dtypes.bfloat16),
            "wtso": wts_o.astype(ml_dtypes.bfloat16),
            "wce": wc_e.astype(ml_dtypes.bfloat16),
            "wco": wc_o.astype(ml_dtypes.bfloat16),
            "bias": bi.reshape(C_OUT, 1),
        }
        for c in range(CORES)
    ]
    return in_maps, tpw, win_free, win_meta, perm


def _assemble(results, tpw, win_meta, perm):
    out_sorted = np.empty((N_VOX, C_OUT), np.float32)
    for c in range(CORES):
        oc = results[c]["out"]
        for w in range(NWIN):
            r0, nr = int(win_meta[c, w, 0]), int(win_meta[c, w, 1])
            c0 = w * tpw * TILE
            out_sorted[r0:r0 + nr] = oc[:, c0:c0 + nr].T
    final = np.empty((N_VOX, C_OUT), np.float32)
    final[perm] = out_sorted
    return final


def kernel(features, coors, weight, bias, batch_size=1, **_kw):
    global LAST_RESULTS
    from concourse.bass_utils import run_bass_kernel_spmd

    in_maps, tpw, win_free, win_meta, perm = _prep(features, coors, weight, bias)
    key = (tpw, win_free)
    if key not in _PROG_CACHE:
        _PROG_CACHE[key] = _build_program(tpw, win_free)
    nc = _PROG_CACHE[key]
    br = run_bass_kernel_spmd(nc, in_maps, list(range(CORES)), trace=TRACE)
    LAST_RESULTS = br
    return _assemble(br.results, tpw, win_meta, perm)
